# revision 9
# baseline (speedup 1.0000x reference)
"""BCQLinear (3-bit binary-coded quantized linear) Trainium2 kernel.

Full-input contract: kernel(**inputs) takes the unsharded inputs of
nn_BCQLinear_88510686036218 and returns the full [1, 128, 4096] output.

Math: w = alpha*(2*S-7) + beta with S in [0,8) the 3-bit code, then
y = (x[:, in_reorder] @ w)[:, out_reorder].

Sharding: out-features split 8 ways (512 cols/core), x replicated.

Hybrid weight path (the key idea): the DVE dequant (bit-extract +
alpha-multiply) runs at ~0.5 ns/element and is the kernel's critical
resource, while the DMA fabric has slack.  So each core's 512 columns
split:
  - PACKED 320 cols (local 0..319): 3-bit codes packed 4-per-int16
    (field r at bits [3r,3r+3), col o' = r*80+u), dequantized on-device:
      extract: vt = W32 & mask(r), int32 TensorScalar per (r, 8-K-tile
               block); scale: wm[p,kt,o'] = vt * a2p, TensorTensor per
               4 K-tiles (the matmul-gating granularity)
    The 8^r field scale rides through the matmul, divided out on host;
    the beta part is a host-side rank-32 correction (corr).
  - DIRECT 192 cols (local 320..511): the host dequantizes fully
    (alpha*(2S-7)+beta, exact in fp16) and streams fp16 weights; no
    DVE work, no beta correction.

DMA discipline (what actually matters on this fabric): the 16 SDMA
engines round-robin between queues at PACKET granularity, so byte
throughput is proportional to descriptor (= dram-row) size, and a
queue's transfers complete in FIFO order.  Hence:
  - ALL packed words + alpha ride ONE early transfer (hwall, 0.72 MB,
    5.8KB rows) that lands ~2.5us in: the whole DVE stream then runs
    gate-free at its own pace.
  - [xt_c | wd_c] merged per chunk into "megas" with multi-KB rows;
    they pace only the matmuls, which trail the dequant anyway.
Other schedule points:
  - Contraction rows band-packed: row i(kt,p) = 128*(p//4)+4*kt+(p%4),
    so one [128,320] alpha tile serves every K-tile.
  - ~20 garbage warm-up matmuls bridge the DMA head so the PE is busy
    through the HAM activity window (cold matmuls run at 1.2 GHz).
  - A (packed) and B (direct) matmuls are paired per K-tile -- same
    stationary xt tile -- accumulating into two separate PSUM banks,
    each padded to a full 2KB bank (PE-write/DVE-read same-bank is
    fatal).
  - No ACT-engine ops (avoids the 1.3us ACT_TABLE_LOAD); output copies
    on DVE; two HWDGE queues only.
"""
import numpy as np
from contextlib import ExitStack

import concourse.bass as bass
import concourse.mybir as mybir
import concourse.tile as tile
from concourse import bacc

IN_F, OUT_F, WBITS, GS, OFI = 4096, 4096, 3, 128, 128
NG, NB = 32, 32
NCORES = 8
OPC = OUT_F // NCORES        # 512 out-cols per core
NPK = 320                    # packed cols per core (local 0..NPK)
NDIR = OPC - NPK             # direct fp16 cols per core
WPF = NPK // 4               # cols per field r (80)
NWC = NPK // 8               # packed int32 words per (partition, K-tile)
NKT = 32                     # K-tiles of 128 rows
NR = 4                       # fields per int16 half
T = 128                      # tokens
M_CHUNKS = [4, 8, 10, 10]    # [xt|wd] mega chunk sizes (K-tiles)
NCHUNK = len(M_CHUNKS)
M_OFF = [sum(M_CHUNKS[:i]) for i in range(NCHUNK)]
EX_BLK = 8                   # K-tiles per extract op
SC_BLK = 4                   # K-tiles per scale piece
N_WARM = 20                  # PE warm-up matmuls (HAM un-throttle)

F32 = mybir.dt.float32
F16 = mybir.dt.float16
I32 = mybir.dt.int32
I16 = mybir.dt.int16
ALU = mybir.AluOpType

_PROGRAM_CACHE = {}


# ---------------------------------------------------------------- host prep
def _dequant_codes(qweight):
    """S[i, o] in [0,8): w = alpha*(2S-7)+beta."""
    qw = np.asarray(qweight, dtype=np.uint32).reshape(NG, NB, WBITS, GS * OFI // 32)
    bits = (qw[..., None] >> np.arange(32, dtype=np.uint32)) & 1
    bits = bits.reshape(NG, NB, WBITS, GS, OFI)
    S = (bits * (2 ** np.arange(WBITS, dtype=np.uint32))[:, None, None]).sum(axis=2)
    return S.transpose(0, 2, 1, 3).reshape(IN_F, OUT_F).astype(np.uint32)


def _band_rows():
    kt, p = np.meshgrid(np.arange(NKT), np.arange(128), indexing="ij")
    return 128 * (p // 4) + 4 * kt + (p % 4)      # [NKT, 128]


def _prepare(inputs):
    x = np.asarray(inputs["x"], np.float32).reshape(-1, IN_F)
    alpha = np.asarray(inputs["alpha"], np.float32)
    beta = np.asarray(inputs["beta"], np.float32)
    in_reorder = np.asarray(inputs["in_reorder"], np.int64)
    xf = x[:, in_reorder]

    S = _dequant_codes(inputs["qweight"])          # [IN_F, OUT_F] uint32
    rows = _band_rows()                            # [NKT, 128]
    rowsf = rows.reshape(-1)
    XT = np.ascontiguousarray(
        xf[:, rowsf].T.reshape(NKT, 128, T).transpose(1, 0, 2)
    ).reshape(128, NKT * T).astype(np.float16)     # [p, kt*T]

    # host-side beta part (packed cols only):
    # corr[t, o] = sum_g xsum[g,t] * (beta-7a)[g,o]
    xsum = xf.reshape(T, NG, GS).sum(axis=2, dtype=np.float64)   # [T, NG]
    Bfull = beta.astype(np.float64) - 7.0 * alpha.astype(np.float64)
    corr = (xsum @ Bfull).astype(np.float32)       # [T, OUT_F]

    g_of_row = rowsf // GS                         # group of each banded row

    in_maps = []
    for c in range(NCORES):
        pk = slice(OPC * c, OPC * c + NPK)         # packed global cols
        dr = slice(OPC * c + NPK, OPC * (c + 1))   # direct global cols
        # packed codes, banded: [p, kt, o']
        Sc = S[rowsf, pk].reshape(NKT, 128, NPK).transpose(1, 0, 2)
        W = np.zeros((128, NKT, NWC), np.uint32)
        for r in range(NR):
            for h in range(2):
                W |= Sc[:, :, r * WPF + h::2][:, :, :NWC] << (3 * r + 16 * h)
        W = W.reshape(128, NKT * NWC)
        a2p = (2.0 * alpha[np.arange(128) // 4][:, pk]).astype(np.float16)
        # direct fp16 weights, banded k-major: wd[p, kt*NDIR + q]
        Sd = S[rowsf, dr].astype(np.float32)       # [NKT*128, NDIR]
        wdf = (alpha[g_of_row][:, dr] * (2.0 * Sd - 7.0)
               + beta[g_of_row][:, dr])            # [NKT*128, NDIR]
        WD = np.ascontiguousarray(
            wdf.reshape(NKT, 128, NDIR).transpose(1, 0, 2)
        ).reshape(128, NKT * NDIR).astype(np.float16)
        im = {}
        # hwall = [a2p | W all kts], int16 rows (the DVE's entire input)
        im["hwall"] = np.ascontiguousarray(np.concatenate(
            [a2p.view(np.int16), W.view(np.int16)], axis=1))
        for ch in range(NCHUNK):
            k0, nk = M_OFF[ch], M_CHUNKS[ch]
            im[f"m{ch}"] = np.ascontiguousarray(np.concatenate(
                [XT[:, k0 * T:(k0 + nk) * T],
                 WD[:, k0 * NDIR:(k0 + nk) * NDIR]], axis=1))
        in_maps.append(im)
    return in_maps, corr


# ---------------------------------------------------------------- program
def build_program():
    nc = bacc.Bacc("TRN2")

    hw_dr = nc.declare_dram_parameter(
        "hwall", [128, NPK + NKT * NWC * 2], I16, isOutput=False)
    m_dr = [nc.declare_dram_parameter(
                f"m{ch}", [128, M_CHUNKS[ch] * (T + NDIR)], F16, isOutput=False)
            for ch in range(NCHUNK)]
    z = nc.declare_dram_parameter("z", [T, OPC], F16, isOutput=True)

    with tile.TileContext(nc) as tc, ExitStack() as ctx:
        cpool = ctx.enter_context(tc.tile_pool(name="const", bufs=1))
        opool = ctx.enter_context(tc.tile_pool(name="out", bufs=1))
        ppool = ctx.enter_context(tc.tile_pool(name="psum", bufs=1, space="PSUM"))

        # --- static tiles --------------------------------------------------
        hw_sb = cpool.tile([128, NPK + NKT * NWC * 2], I16, tag="hwall")
        m_sb = [cpool.tile([128, M_CHUNKS[ch] * (T + NDIR)], F16, tag=f"m{ch}",
                           name=f"msb{ch}")
                for ch in range(NCHUNK)]
        a2_v = hw_sb[:, :NPK].bitcast(F16)

        def xt_v(ch):
            return m_sb[ch][:, :M_CHUNKS[ch] * T]

        def wd_v(ch):
            return m_sb[ch][:, M_CHUNKS[ch] * T:]

        vt = cpool.tile([128, NKT * NPK], I16, tag="vt")
        wm = cpool.tile([128, NKT * NPK], F16, tag="wm")
        gw = cpool.tile([128, 128], F16, tag="gw")     # warm-up operand

        # each psum tile padded to a full 2KB bank (collision safety)
        psA = ppool.tile([T, 512], F32, tag="psA")
        psB = ppool.tile([T, 512], F32, tag="psB")
        psW = ppool.tile([T, 512], F32, tag="psW")     # warm-up target

        # --- DMA schedule: FIFO per queue == completion order -------------
        nc.sync.dma_start(out=hw_sb[:], in_=hw_dr[:])
        nc.scalar.dma_start(out=m_sb[0][:], in_=m_dr[0][:])
        nc.sync.dma_start(out=m_sb[1][:], in_=m_dr[1][:])
        nc.scalar.dma_start(out=m_sb[2][:], in_=m_dr[2][:])
        nc.sync.dma_start(out=m_sb[3][:], in_=m_dr[3][:])

        # --- PE warm-up: keep the array busy through the HAM window -------
        nc.gpsimd.memset(gw[:], 0.0)
        for _ in range(N_WARM):
            nc.tensor.matmul(psW[:, :128], gw[:], gw[:], start=True, stop=True)

        # --- dequant pass 1: vt[p, r, kt, u] = W32 & (7<<3r both halves) --
        def extract(b, r):
            k0 = b * EX_BLK
            src = hw_sb[:, NPK + k0 * NWC * 2:
                        NPK + (k0 + EX_BLK) * NWC * 2].bitcast(I32)
            dst = vt[:, r * NKT * WPF + k0 * WPF:
                     r * NKT * WPF + (k0 + EX_BLK) * WPF].bitcast(I32)
            m = 7 << (3 * r)
            nc.vector.tensor_scalar(dst, src, (m << 16) | m, None,
                                    ALU.bitwise_and)

        # --- dequant pass 2 (piece): wm[p,kt,o'] = vt * a2p, kts [k0,k1) --
        def scale(k0, k1):
            npc = k1 - k0
            in0 = vt[:].rearrange("p (r k u) -> p k r u", r=NR,
                                  u=WPF)[:, k0:k1]
            out = wm[:].rearrange("p (k r u) -> p k r u", r=NR,
                                  u=WPF)[:, k0:k1]
            in1 = a2_v.rearrange("p (r u) -> p r u", r=NR).unsqueeze(
                1).broadcast_to([128, npc, NR, WPF])
            nc.vector.tensor_tensor(out, in0, in1, ALU.mult)

        def chunk_of(kt):
            for ch in range(NCHUNK):
                if M_OFF[ch] <= kt < M_OFF[ch] + M_CHUNKS[ch]:
                    return ch
            raise AssertionError

        def mmA(kt):
            ch = chunk_of(kt)
            kl = kt - M_OFF[ch]
            nc.tensor.matmul(
                psA[:, :NPK],
                xt_v(ch)[:, kl * T:(kl + 1) * T],
                wm[:, kt * NPK:(kt + 1) * NPK],
                start=(kt == 0), stop=(kt == NKT - 1))

        def mmB(kt):
            ch = chunk_of(kt)
            kl = kt - M_OFF[ch]
            nc.tensor.matmul(
                psB[:, :NDIR],
                xt_v(ch)[:, kl * T:(kl + 1) * T],
                wd_v(ch)[:, kl * NDIR:(kl + 1) * NDIR],
                start=(kt == 0), stop=(kt == NKT - 1))

        out_a = opool.tile([T, NPK], F16, tag="out_a")
        out_b = opool.tile([T, NDIR], F16, tag="out_b")

        for b in range(NKT // EX_BLK):
            for r in range(NR):
                extract(b, r)
            for k0 in range(b * EX_BLK, (b + 1) * EX_BLK, SC_BLK):
                scale(k0, k0 + SC_BLK)
                for kt in range(k0, k0 + SC_BLK):
                    mmA(kt)
                    mmB(kt)
        nc.vector.tensor_copy(out_a[:], psA[:, :NPK])
        nc.sync.dma_start(out=z[:, :NPK], in_=out_a[:])
        nc.vector.tensor_copy(out_b[:], psB[:, :NDIR])
        nc.scalar.dma_start(out=z[:, NPK:], in_=out_b[:])
    nc.finalize()
    return nc


def _get_program():
    if "nc" not in _PROGRAM_CACHE:
        _PROGRAM_CACHE["nc"] = build_program()
    return _PROGRAM_CACHE["nc"]


# ---------------------------------------------------------------- entry
def kernel(**inputs):
    from concourse.bass_utils import run_bass_kernel_spmd

    in_maps, corr = _prepare(inputs)
    nc = _get_program()
    res = run_bass_kernel_spmd(nc, in_maps, list(range(NCORES)))
    out_reorder = np.asarray(inputs["out_reorder"], np.int64)
    # per-core: col j<NPK is packed field r=j//WPF (descale 8^-r, add corr);
    # col j>=NPK is direct (exact, no correction)
    rs = np.concatenate([np.repeat(8.0 ** -np.arange(NR), WPF),
                         np.ones(NDIR, np.float64)]).astype(np.float32)
    y = np.empty((T, OUT_F), np.float32)
    for c in range(NCORES):
        zc = res.results[c]["z"].astype(np.float32) * rs[None, :]
        zc[:, :NPK] += corr[:, OPC * c:OPC * c + NPK]
        y[:, OPC * c:OPC * (c + 1)] = zc
    y = y[:, out_reorder]
    return y.reshape(1, T, OUT_F).astype(np.float32)


# revision 10
# speedup vs baseline: 1.0593x; 1.0593x over previous
"""BCQLinear (3-bit binary-coded quantized linear) Trainium2 kernel.

Full-input contract: kernel(**inputs) takes the unsharded inputs of
nn_BCQLinear_88510686036218 and returns the full [1, 128, 4096] output.

Math: w = alpha*(2*S-7) + beta with S in [0,8) the 3-bit code, then
y = (x[:, in_reorder] @ w)[:, out_reorder].

Sharding: out-features split 8 ways (512 cols/core), x replicated.

Hybrid weight path (the key idea): the DVE dequant (bit-extract +
alpha-multiply) runs at ~0.5 ns/element and is the kernel's critical
resource, while the DMA fabric has slack.  So each core's 512 columns
split:
  - PACKED 320 cols (local 0..319): 3-bit codes packed 4-per-int16
    (field r at bits [3r,3r+3), col o' = r*80+u), dequantized on-device:
      extract: vt = W32 & mask(r), one int32 TensorScalar per (chunk,r)
      scale:   wm[p,kt,o'] = vt * a2p, TensorTensor per 4-6 K-tiles
    The 8^r field scale rides through the matmul, divided out on host;
    the beta part is a host-side rank-32 correction (corr).
  - DIRECT 192 cols (local 320..511): the host dequantizes fully
    (alpha*(2S-7)+beta, exact in fp16) and streams fp16 weights; no
    DVE work, no beta correction.

DMA discipline (what actually matters on this fabric): the 16 SDMA
engines round-robin between queues at PACKET granularity, so byte
throughput is proportional to descriptor (= dram-row) size, and each
queue's transfers complete in FIFO order.  Both queues carry a mix of
small (w) and large ([xt|wd] mega) rows ordered by consumer deadline:
the [alpha|w0] head piece lands ~2.8us in and starts the dequant; the
last chunk's wd3 is split out and scheduled dead last -- it only gates
the final 6 direct matmuls, minimizing the post-DMA tail.

Other schedule points:
  - Contraction rows band-packed: row i(kt,p) = 128*(p//4)+4*kt+(p%4),
    so one [128,320] alpha tile serves every K-tile.
  - ~26 garbage warm-up matmuls bridge the DMA head so the PE is busy
    through the HAM activity window (cold matmuls run at half clock).
  - A (packed) and B (direct) matmuls are paired per K-tile (same
    stationary xt), keeping the PE stream dense so HAM stays warm;
    separate PSUM accumulators padded to full 2KB banks (PE-write +
    DVE-read in one bank is fatal).
  - No ACT-engine ops (avoids the 1.3us ACT_TABLE_LOAD); output copies
    on DVE; two HWDGE queues only.
"""
import numpy as np
from contextlib import ExitStack

import concourse.bass as bass
import concourse.mybir as mybir
import concourse.tile as tile
from concourse import bacc

IN_F, OUT_F, WBITS, GS, OFI = 4096, 4096, 3, 128, 128
NG, NB = 32, 32
NCORES = 8
OPC = OUT_F // NCORES        # 512 out-cols per core
NPK = 320                    # packed cols per core (local 0..NPK)
NDIR = OPC - NPK             # direct fp16 cols per core
WPF = NPK // 4               # cols per field r (80)
NWC = NPK // 8               # packed int32 words per (partition, K-tile)
NKT = 32                     # K-tiles of 128 rows
NR = 4                       # fields per int16 half
T = 128                      # tokens
KT_CHUNKS = [4, 12, 10, 6]   # pipeline chunk sizes (K-tiles)
NCHUNK = len(KT_CHUNKS)
KT_OFF = [sum(KT_CHUNKS[:i]) for i in range(NCHUNK)]
SCALE_PIECES = {0: [4], 1: [4, 4, 4], 2: [5, 5], 3: [3, 3]}
N_WARM = 26                  # PE warm-up matmuls (HAM un-throttle)

F32 = mybir.dt.float32
F16 = mybir.dt.float16
I32 = mybir.dt.int32
I16 = mybir.dt.int16
ALU = mybir.AluOpType

_PROGRAM_CACHE = {}


# ---------------------------------------------------------------- host prep
def _dequant_codes(qweight):
    """S[i, o] in [0,8): w = alpha*(2S-7)+beta."""
    qw = np.asarray(qweight, dtype=np.uint32).reshape(NG, NB, WBITS, GS * OFI // 32)
    bits = (qw[..., None] >> np.arange(32, dtype=np.uint32)) & 1
    bits = bits.reshape(NG, NB, WBITS, GS, OFI)
    S = (bits * (2 ** np.arange(WBITS, dtype=np.uint32))[:, None, None]).sum(axis=2)
    return S.transpose(0, 2, 1, 3).reshape(IN_F, OUT_F).astype(np.uint32)


def _band_rows():
    kt, p = np.meshgrid(np.arange(NKT), np.arange(128), indexing="ij")
    return 128 * (p // 4) + 4 * kt + (p % 4)      # [NKT, 128]


def _prepare(inputs):
    x = np.asarray(inputs["x"], np.float32).reshape(-1, IN_F)
    alpha = np.asarray(inputs["alpha"], np.float32)
    beta = np.asarray(inputs["beta"], np.float32)
    in_reorder = np.asarray(inputs["in_reorder"], np.int64)
    xf = x[:, in_reorder]

    S = _dequant_codes(inputs["qweight"])          # [IN_F, OUT_F] uint32
    rows = _band_rows()                            # [NKT, 128]
    rowsf = rows.reshape(-1)
    XT = np.ascontiguousarray(
        xf[:, rowsf].T.reshape(NKT, 128, T).transpose(1, 0, 2)
    ).reshape(128, NKT * T).astype(np.float16)     # [p, kt*T]

    # host-side beta part (packed cols only):
    # corr[t, o] = sum_g xsum[g,t] * (beta-7a)[g,o]
    xsum = xf.reshape(T, NG, GS).sum(axis=2, dtype=np.float64)   # [T, NG]
    Bfull = beta.astype(np.float64) - 7.0 * alpha.astype(np.float64)
    corr = (xsum @ Bfull).astype(np.float32)       # [T, OUT_F]

    g_of_row = rowsf // GS                         # group of each banded row

    in_maps = []
    for c in range(NCORES):
        pk = slice(OPC * c, OPC * c + NPK)         # packed global cols
        dr = slice(OPC * c + NPK, OPC * (c + 1))   # direct global cols
        # packed codes, banded: [p, kt, o']
        Sc = S[rowsf, pk].reshape(NKT, 128, NPK).transpose(1, 0, 2)
        W = np.zeros((128, NKT, NWC), np.uint32)
        for r in range(NR):
            for h in range(2):
                W |= Sc[:, :, r * WPF + h::2][:, :, :NWC] << (3 * r + 16 * h)
        W = W.reshape(128, NKT * NWC)
        a2p = (2.0 * alpha[np.arange(128) // 4][:, pk]).astype(np.float16)
        # direct fp16 weights, banded k-major: wd[p, kt*NDIR + q]
        Sd = S[rowsf, dr].astype(np.float32)       # [NKT*128, NDIR]
        wdf = (alpha[g_of_row][:, dr] * (2.0 * Sd - 7.0)
               + beta[g_of_row][:, dr])            # [NKT*128, NDIR]
        WD = np.ascontiguousarray(
            wdf.reshape(NKT, 128, NDIR).transpose(1, 0, 2)
        ).reshape(128, NKT * NDIR).astype(np.float16)
        im = {}
        # hw0 = [a2p | w chunk 0], int16 rows
        nk0 = KT_CHUNKS[0]
        im["hw0"] = np.ascontiguousarray(np.concatenate(
            [a2p.view(np.int16),
             np.ascontiguousarray(W[:, :nk0 * NWC]).view(np.int16)], axis=1))
        for ch in range(1, NCHUNK):
            k0, nk = KT_OFF[ch], KT_CHUNKS[ch]
            im[f"w{ch}"] = np.ascontiguousarray(
                W[:, k0 * NWC:(k0 + nk) * NWC]).view(np.int32)
        for ch in range(NCHUNK - 1):
            k0, nk = KT_OFF[ch], KT_CHUNKS[ch]
            im[f"m{ch}"] = np.ascontiguousarray(np.concatenate(
                [XT[:, k0 * T:(k0 + nk) * T],
                 WD[:, k0 * NDIR:(k0 + nk) * NDIR]], axis=1))
        k0, nk = KT_OFF[-1], KT_CHUNKS[-1]
        im["xt3"] = np.ascontiguousarray(XT[:, k0 * T:(k0 + nk) * T])
        im["wd3"] = np.ascontiguousarray(WD[:, k0 * NDIR:(k0 + nk) * NDIR])
        in_maps.append(im)
    return in_maps, corr


# ---------------------------------------------------------------- program
def build_program():
    nc = bacc.Bacc("TRN2")

    hw0_dr = nc.declare_dram_parameter(
        "hw0", [128, NPK + KT_CHUNKS[0] * NWC * 2], I16, isOutput=False)
    w_dr = {ch: nc.declare_dram_parameter(
                f"w{ch}", [128, KT_CHUNKS[ch] * NWC], I32, isOutput=False)
            for ch in range(1, NCHUNK)}
    m_dr = [nc.declare_dram_parameter(
                f"m{ch}", [128, KT_CHUNKS[ch] * (T + NDIR)], F16, isOutput=False)
            for ch in range(NCHUNK - 1)]
    xt3_dr = nc.declare_dram_parameter(
        "xt3", [128, KT_CHUNKS[-1] * T], F16, isOutput=False)
    wd3_dr = nc.declare_dram_parameter(
        "wd3", [128, KT_CHUNKS[-1] * NDIR], F16, isOutput=False)
    z = nc.declare_dram_parameter("z", [T, OPC], F16, isOutput=True)

    with tile.TileContext(nc) as tc, ExitStack() as ctx:
        cpool = ctx.enter_context(tc.tile_pool(name="const", bufs=1))
        opool = ctx.enter_context(tc.tile_pool(name="out", bufs=1))
        ppool = ctx.enter_context(tc.tile_pool(name="psum", bufs=1, space="PSUM"))

        # --- static tiles --------------------------------------------------
        hw0_sb = cpool.tile([128, NPK + KT_CHUNKS[0] * NWC * 2], I16, tag="hw0")
        w_sb = {ch: cpool.tile([128, KT_CHUNKS[ch] * NWC], I32, tag=f"w{ch}",
                               name=f"wsb{ch}")
                for ch in range(1, NCHUNK)}
        m_sb = [cpool.tile([128, KT_CHUNKS[ch] * (T + NDIR)], F16, tag=f"m{ch}",
                           name=f"msb{ch}")
                for ch in range(NCHUNK - 1)]
        xt3_sb = cpool.tile([128, KT_CHUNKS[-1] * T], F16, tag="xt3")
        wd3_sb = cpool.tile([128, KT_CHUNKS[-1] * NDIR], F16, tag="wd3")
        a2_v = hw0_sb[:, :NPK].bitcast(F16)
        w0_v = hw0_sb[:, NPK:].bitcast(I32)

        def xt_v(ch):
            if ch == NCHUNK - 1:
                return xt3_sb[:]
            return m_sb[ch][:, :KT_CHUNKS[ch] * T]

        def wd_v(ch):
            if ch == NCHUNK - 1:
                return wd3_sb[:]
            return m_sb[ch][:, KT_CHUNKS[ch] * T:]

        vt = [cpool.tile([128, KT_CHUNKS[ch] * NPK], I16, tag=f"vt{ch}",
                         name=f"vtt{ch}")
              for ch in range(NCHUNK)]
        wm = [cpool.tile([128, KT_CHUNKS[ch] * NPK], F16, tag=f"wm{ch}",
                         name=f"wmt{ch}")
              for ch in range(NCHUNK)]
        gw = cpool.tile([128, 128], F16, tag="gw")     # warm-up operand

        # each psum tile padded to a full 2KB bank (collision safety)
        psA = ppool.tile([T, 512], F32, tag="psA")
        psB = ppool.tile([T, 512], F32, tag="psB")
        psW = ppool.tile([T, 512], F32, tag="psW")     # warm-up target

        # --- DMA schedule: FIFO per queue, ordered by consumer deadline ---
        nc.sync.dma_start(out=hw0_sb[:], in_=hw0_dr[:])
        nc.scalar.dma_start(out=w_sb[1][:], in_=w_dr[1][:])
        nc.sync.dma_start(out=m_sb[0][:], in_=m_dr[0][:])
        nc.scalar.dma_start(out=w_sb[2][:], in_=w_dr[2][:])
        nc.sync.dma_start(out=m_sb[1][:], in_=m_dr[1][:])
        nc.scalar.dma_start(out=m_sb[2][:], in_=m_dr[2][:])
        nc.sync.dma_start(out=w_sb[3][:], in_=w_dr[3][:])
        nc.sync.dma_start(out=xt3_sb[:], in_=xt3_dr[:])
        nc.scalar.dma_start(out=wd3_sb[:], in_=wd3_dr[:])

        # --- PE warm-up: keep the array busy through the HAM window -------
        nc.gpsimd.memset(gw[:], 0.0)
        for _ in range(N_WARM):
            nc.tensor.matmul(psW[:, :128], gw[:], gw[:], start=True, stop=True)

        # --- dequant pass 1: vt[p, r, kt, u] = W32 & (7<<3r both halves) --
        def extract(ch, r):
            nk = KT_CHUNKS[ch]
            src = w0_v if ch == 0 else w_sb[ch][:]
            m = 7 << (3 * r)
            nc.vector.tensor_scalar(
                vt[ch][:, r * nk * WPF:(r + 1) * nk * WPF].bitcast(I32),
                src, (m << 16) | m, None, ALU.bitwise_and)

        # --- dequant pass 2 (piece): wm[p,kt,o'] = vt * a2p, kts [k0,k1) --
        def scale(ch, k0, k1):
            npc = k1 - k0
            in0 = vt[ch][:].rearrange("p (r k u) -> p k r u", r=NR,
                                      u=WPF)[:, k0:k1]
            out = wm[ch][:].rearrange("p (k r u) -> p k r u", r=NR,
                                      u=WPF)[:, k0:k1]
            in1 = a2_v.rearrange("p (r u) -> p r u", r=NR).unsqueeze(
                1).broadcast_to([128, npc, NR, WPF])
            nc.vector.tensor_tensor(out, in0, in1, ALU.mult)

        def mmA(ch, kl):
            kt = KT_OFF[ch] + kl
            nc.tensor.matmul(
                psA[:, :NPK],
                xt_v(ch)[:, kl * T:(kl + 1) * T],
                wm[ch][:, kl * NPK:(kl + 1) * NPK],
                start=(kt == 0), stop=(kt == NKT - 1))

        def mmB(ch, kl):
            kt = KT_OFF[ch] + kl
            nc.tensor.matmul(
                psB[:, :NDIR],
                xt_v(ch)[:, kl * T:(kl + 1) * T],
                wd_v(ch)[:, kl * NDIR:(kl + 1) * NDIR],
                start=(kt == 0), stop=(kt == NKT - 1))

        out_a = opool.tile([T, NPK], F16, tag="out_a")
        out_b = opool.tile([T, NDIR], F16, tag="out_b")

        for ch in range(NCHUNK):
            for r in range(NR):
                extract(ch, r)
            k0 = 0
            for npc in SCALE_PIECES[ch]:
                scale(ch, k0, k0 + npc)
                for kl in range(k0, k0 + npc):
                    mmA(ch, kl)
                    mmB(ch, kl)
                k0 += npc
        # A finishes first (DVE-gated); B's wd3 lands last by design
        nc.vector.tensor_copy(out_a[:], psA[:, :NPK])
        nc.sync.dma_start(out=z[:, :NPK], in_=out_a[:])
        nc.vector.tensor_copy(out_b[:], psB[:, :NDIR])
        nc.scalar.dma_start(out=z[:, NPK:], in_=out_b[:])
    nc.finalize()
    return nc


def _get_program():
    if "nc" not in _PROGRAM_CACHE:
        _PROGRAM_CACHE["nc"] = build_program()
    return _PROGRAM_CACHE["nc"]


# ---------------------------------------------------------------- entry
def kernel(**inputs):
    from concourse.bass_utils import run_bass_kernel_spmd

    in_maps, corr = _prepare(inputs)
    nc = _get_program()
    res = run_bass_kernel_spmd(nc, in_maps, list(range(NCORES)))
    out_reorder = np.asarray(inputs["out_reorder"], np.int64)
    # per-core: col j<NPK is packed field r=j//WPF (descale 8^-r, add corr);
    # col j>=NPK is direct (exact, no correction)
    rs = np.concatenate([np.repeat(8.0 ** -np.arange(NR), WPF),
                         np.ones(NDIR, np.float64)]).astype(np.float32)
    y = np.empty((T, OUT_F), np.float32)
    for c in range(NCORES):
        zc = res.results[c]["z"].astype(np.float32) * rs[None, :]
        zc[:, :NPK] += corr[:, OPC * c:OPC * c + NPK]
        y[:, OPC * c:OPC * (c + 1)] = zc
    y = y[:, out_reorder]
    return y.reshape(1, T, OUT_F).astype(np.float32)


# revision 11
# speedup vs baseline: 1.1359x; 1.0723x over previous
"""BCQLinear (3-bit binary-coded quantized linear) Trainium2 kernel.

Full-input contract: kernel(**inputs) takes the unsharded inputs of
nn_BCQLinear_88510686036218 and returns the full [1, 128, 4096] output.

Math: w = alpha*(2*S-7) + beta with S in [0,8) the 3-bit code, then
y = (x[:, in_reorder] @ w)[:, out_reorder].

Sharding: out-features split 8 ways (512 cols/core), x replicated.

Hybrid weight path (the key idea): the DVE dequant (bit-extract +
alpha-multiply) runs at ~0.5 ns/element and is the kernel's critical
resource, while the DMA fabric has slack.  So each core's 512 columns
split:
  - PACKED 320 cols (local 0..319): 3-bit codes packed 4-per-int16
    (field r at bits [3r,3r+3), col o' = r*80+u), dequantized on-device:
      extract: vt = W32 & mask(r), one int32 TensorScalar per (chunk,r)
      scale:   wm[p,kt,o'] = vt * a2p, TensorTensor per ~4-5 K-tiles
    The 8^r field scale rides through the matmul, divided out on host;
    the beta part is a host-side rank-32 correction (corr).
  - DIRECT 192 cols (local 320..511): the host dequantizes fully
    (alpha*(2S-7)+beta, exact in fp16) and streams fp16 weights; no
    DVE work, no beta correction.

DMA discipline (what actually matters on this fabric): the 16 SDMA
engines round-robin between queues at PACKET granularity, so byte
throughput is proportional to descriptor (= dram-row) size; small-row
transfers starve next to big-row ones, and a queue's transfers complete
in FIFO order.  Hence:
  - [a2p | w0] are merged into one early small transfer (the DVE's
    critical input), [xt_c | wd_c] are merged per chunk into one
    "mega" with uniform multi-KB rows, and the w_c extracts ride their
    own small transfers ordered ahead of the megas they beat.
  - chunks [4,10,12,6]: first chunk small (dequant starts ~2.5us in),
    last chunk small (its mega lands last; short matmul tail).

Other schedule points:
  - Contraction rows band-packed: row i(kt,p) = 128*(p//4)+4*kt+(p%4),
    so one [128,320] alpha tile serves every K-tile.
  - ~30 garbage warm-up matmuls bridge the DMA head so the PE is busy
    through the HAM activity window (real matmuls then run at 2.4 GHz,
    not the cold 1.2 GHz clock gate).
  - Per chunk: A-matmuls (packed, gated on DVE scale pieces) are
    emitted before B-matmuls (direct, gated on the chunk's mega DMA),
    matching their expected ready times (PE executes in order).
  - psA/psB are padded to full 2KB PSUM banks so PE writes and DVE
    reads never share a bank (collision is fatal).
  - No ACT-engine ops (avoids the 1.3us ACT_TABLE_LOAD on the scalar
    queue); output copies on DVE.
"""
import numpy as np
from contextlib import ExitStack

import concourse.bass as bass
import concourse.mybir as mybir
import concourse.tile as tile
from concourse import bacc

IN_F, OUT_F, WBITS, GS, OFI = 4096, 4096, 3, 128, 128
NG, NB = 32, 32
NCORES = 8
OPC = OUT_F // NCORES        # 512 out-cols per core
NPK = 320                    # packed cols per core (local 0..NPK)
NDIR = OPC - NPK             # direct fp16 cols per core
WPF = NPK // 4               # cols per field r (80)
NWC = NPK // 8               # packed int32 words per (partition, K-tile)
NKT = 32                     # K-tiles of 128 rows
NR = 4                       # fields per int16 half
T = 128                      # tokens
KT_CHUNKS = [4, 10, 12, 6]   # DMA pipeline chunk sizes (K-tiles)
NCHUNK = len(KT_CHUNKS)
KT_OFF = [sum(KT_CHUNKS[:i]) for i in range(NCHUNK)]
SCALE_PIECES = {0: [4], 1: [5, 5], 2: [4, 4, 4], 3: [3, 3]}
N_WARM = 30                  # PE warm-up matmuls (HAM un-throttle)

F32 = mybir.dt.float32
F16 = mybir.dt.float16
I32 = mybir.dt.int32
I16 = mybir.dt.int16
ALU = mybir.AluOpType

_PROGRAM_CACHE = {}


# ---------------------------------------------------------------- host prep
def _dequant_codes(qweight):
    """S[i, o] in [0,8): w = alpha*(2S-7)+beta."""
    qw = np.asarray(qweight, dtype=np.uint32).reshape(NG, NB, WBITS, GS * OFI // 32)
    bits = (qw[..., None] >> np.arange(32, dtype=np.uint32)) & 1
    bits = bits.reshape(NG, NB, WBITS, GS, OFI)
    S = (bits * (2 ** np.arange(WBITS, dtype=np.uint32))[:, None, None]).sum(axis=2)
    return S.transpose(0, 2, 1, 3).reshape(IN_F, OUT_F).astype(np.uint32)


def _band_rows():
    kt, p = np.meshgrid(np.arange(NKT), np.arange(128), indexing="ij")
    return 128 * (p // 4) + 4 * kt + (p % 4)      # [NKT, 128]


def _prepare(inputs):
    x = np.asarray(inputs["x"], np.float32).reshape(-1, IN_F)
    alpha = np.asarray(inputs["alpha"], np.float32)
    beta = np.asarray(inputs["beta"], np.float32)
    in_reorder = np.asarray(inputs["in_reorder"], np.int64)
    xf = x[:, in_reorder]

    S = _dequant_codes(inputs["qweight"])          # [IN_F, OUT_F] uint32
    rows = _band_rows()                            # [NKT, 128]
    rowsf = rows.reshape(-1)
    XT = np.ascontiguousarray(
        xf[:, rowsf].T.reshape(NKT, 128, T).transpose(1, 0, 2)
    ).reshape(128, NKT * T).astype(np.float16)     # [p, kt*T]

    # host-side beta part (packed cols only):
    # corr[t, o] = sum_g xsum[g,t] * (beta-7a)[g,o]
    xsum = xf.reshape(T, NG, GS).sum(axis=2, dtype=np.float64)   # [T, NG]
    Bfull = beta.astype(np.float64) - 7.0 * alpha.astype(np.float64)
    corr = (xsum @ Bfull).astype(np.float32)       # [T, OUT_F]

    g_of_row = rowsf // GS                         # group of each banded row

    in_maps = []
    for c in range(NCORES):
        pk = slice(OPC * c, OPC * c + NPK)         # packed global cols
        dr = slice(OPC * c + NPK, OPC * (c + 1))   # direct global cols
        # packed codes, banded: [p, kt, o']
        Sc = S[rowsf, pk].reshape(NKT, 128, NPK).transpose(1, 0, 2)
        W = np.zeros((128, NKT, NWC), np.uint32)
        for r in range(NR):
            for h in range(2):
                W |= Sc[:, :, r * WPF + h::2][:, :, :NWC] << (3 * r + 16 * h)
        W = W.reshape(128, NKT * NWC)
        a2p = (2.0 * alpha[np.arange(128) // 4][:, pk]).astype(np.float16)
        # direct fp16 weights, banded k-major: wd[p, kt*NDIR + q]
        Sd = S[rowsf, dr].astype(np.float32)       # [NKT*128, NDIR]
        wdf = (alpha[g_of_row][:, dr] * (2.0 * Sd - 7.0)
               + beta[g_of_row][:, dr])            # [NKT*128, NDIR]
        WD = np.ascontiguousarray(
            wdf.reshape(NKT, 128, NDIR).transpose(1, 0, 2)
        ).reshape(128, NKT * NDIR).astype(np.float16)
        im = {}
        # hw0 = [a2p | w chunk 0], int16 rows
        k0, nk = KT_OFF[0], KT_CHUNKS[0]
        im["hw0"] = np.ascontiguousarray(np.concatenate(
            [a2p.view(np.int16),
             np.ascontiguousarray(W[:, :nk * NWC]).view(np.int16)], axis=1))
        for ch in range(1, NCHUNK):
            k0, nk = KT_OFF[ch], KT_CHUNKS[ch]
            im[f"w{ch}"] = np.ascontiguousarray(
                W[:, k0 * NWC:(k0 + nk) * NWC]).view(np.int32)
        for ch in range(NCHUNK):
            k0, nk = KT_OFF[ch], KT_CHUNKS[ch]
            im[f"m{ch}"] = np.ascontiguousarray(np.concatenate(
                [XT[:, k0 * T:(k0 + nk) * T],
                 WD[:, k0 * NDIR:(k0 + nk) * NDIR]], axis=1))
        in_maps.append(im)
    return in_maps, corr


# ---------------------------------------------------------------- program
def build_program():
    nc = bacc.Bacc("TRN2")

    hw0_dr = nc.declare_dram_parameter(
        "hw0", [128, NPK + KT_CHUNKS[0] * NWC * 2], I16, isOutput=False)
    w_dr = {ch: nc.declare_dram_parameter(
                f"w{ch}", [128, KT_CHUNKS[ch] * NWC], I32, isOutput=False)
            for ch in range(1, NCHUNK)}
    m_dr = [nc.declare_dram_parameter(
                f"m{ch}", [128, KT_CHUNKS[ch] * (T + NDIR)], F16, isOutput=False)
            for ch in range(NCHUNK)]
    z = nc.declare_dram_parameter("z", [T, OPC], F16, isOutput=True)

    with tile.TileContext(nc) as tc, ExitStack() as ctx:
        cpool = ctx.enter_context(tc.tile_pool(name="const", bufs=1))
        opool = ctx.enter_context(tc.tile_pool(name="out", bufs=1))
        ppool = ctx.enter_context(tc.tile_pool(name="psum", bufs=1, space="PSUM"))

        # --- static tiles --------------------------------------------------
        hw0_sb = cpool.tile([128, NPK + KT_CHUNKS[0] * NWC * 2], I16, tag="hw0")
        w_sb = {ch: cpool.tile([128, KT_CHUNKS[ch] * NWC], I32, tag=f"w{ch}",
                               name=f"wsb{ch}")
                for ch in range(1, NCHUNK)}
        m_sb = [cpool.tile([128, KT_CHUNKS[ch] * (T + NDIR)], F16, tag=f"m{ch}",
                           name=f"msb{ch}")
                for ch in range(NCHUNK)]
        a2_v = hw0_sb[:, :NPK].bitcast(F16)
        w0_v = hw0_sb[:, NPK:].bitcast(I32)

        def xt_v(ch):
            return m_sb[ch][:, :KT_CHUNKS[ch] * T]

        def wd_v(ch):
            return m_sb[ch][:, KT_CHUNKS[ch] * T:]

        vt = [cpool.tile([128, KT_CHUNKS[ch] * NPK], I16, tag=f"vt{ch}",
                         name=f"vtt{ch}")
              for ch in range(NCHUNK)]
        wm = [cpool.tile([128, KT_CHUNKS[ch] * NPK], F16, tag=f"wm{ch}",
                         name=f"wmt{ch}")
              for ch in range(NCHUNK)]
        gw = cpool.tile([128, 128], F16, tag="gw")     # warm-up operand

        # each psum tile padded to a full 2KB bank (collision safety)
        psA = ppool.tile([T, 512], F32, tag="psA")
        psB = ppool.tile([T, 512], F32, tag="psB")
        psW = ppool.tile([T, 512], F32, tag="psW")     # warm-up target

        # --- DMA schedule: FIFO per queue == completion order -------------
        nc.sync.dma_start(out=hw0_sb[:], in_=hw0_dr[:])
        nc.scalar.dma_start(out=w_sb[1][:], in_=w_dr[1][:])
        nc.sync.dma_start(out=m_sb[0][:], in_=m_dr[0][:])
        nc.scalar.dma_start(out=w_sb[2][:], in_=w_dr[2][:])
        nc.scalar.dma_start(out=w_sb[3][:], in_=w_dr[3][:])
        nc.sync.dma_start(out=m_sb[1][:], in_=m_dr[1][:])
        nc.scalar.dma_start(out=m_sb[2][:], in_=m_dr[2][:])
        nc.scalar.dma_start(out=m_sb[3][:], in_=m_dr[3][:])

        # --- PE warm-up: keep the array busy through the HAM window -------
        nc.gpsimd.memset(gw[:], 0.0)
        for _ in range(N_WARM):
            nc.tensor.matmul(psW[:, :128], gw[:], gw[:], start=True, stop=True)

        # --- dequant pass 1: vt[p, r, kt, u] = W32 & (7<<3r both halves) --
        def extract(ch, r):
            nk = KT_CHUNKS[ch]
            src = w0_v if ch == 0 else w_sb[ch][:]
            m = 7 << (3 * r)
            nc.vector.tensor_scalar(
                vt[ch][:, r * nk * WPF:(r + 1) * nk * WPF].bitcast(I32),
                src, (m << 16) | m, None, ALU.bitwise_and)

        # --- dequant pass 2 (piece): wm[p,kt,o'] = vt * a2p, kts [k0,k1) --
        def scale(ch, k0, k1):
            npc = k1 - k0
            in0 = vt[ch][:].rearrange("p (r k u) -> p k r u", r=NR,
                                      u=WPF)[:, k0:k1]
            out = wm[ch][:].rearrange("p (k r u) -> p k r u", r=NR,
                                      u=WPF)[:, k0:k1]
            in1 = a2_v.rearrange("p (r u) -> p r u", r=NR).unsqueeze(
                1).broadcast_to([128, npc, NR, WPF])
            nc.vector.tensor_tensor(out, in0, in1, ALU.mult)

        def mmA(ch, kl):
            kt = KT_OFF[ch] + kl
            nc.tensor.matmul(
                psA[:, :NPK],
                xt_v(ch)[:, kl * T:(kl + 1) * T],
                wm[ch][:, kl * NPK:(kl + 1) * NPK],
                start=(kt == 0), stop=(kt == NKT - 1))

        def mmB(ch, kl):
            kt = KT_OFF[ch] + kl
            nc.tensor.matmul(
                psB[:, :NDIR],
                xt_v(ch)[:, kl * T:(kl + 1) * T],
                wd_v(ch)[:, kl * NDIR:(kl + 1) * NDIR],
                start=(kt == 0), stop=(kt == NKT - 1))

        out_a = opool.tile([T, NPK], F16, tag="out_a")
        out_b = opool.tile([T, NDIR], F16, tag="out_b")

        for ch in range(NCHUNK):
            nk = KT_CHUNKS[ch]
            for r in range(NR):
                extract(ch, r)
            k0 = 0
            for npc in SCALE_PIECES[ch]:
                scale(ch, k0, k0 + npc)
                for kl in range(k0, k0 + npc):
                    mmA(ch, kl)
                k0 += npc
            for kl in range(nk):
                mmB(ch, kl)
        # A finishes first (DVE-gated); B's last mega lands last
        nc.vector.tensor_copy(out_a[:], psA[:, :NPK])
        nc.sync.dma_start(out=z[:, :NPK], in_=out_a[:])
        nc.vector.tensor_copy(out_b[:], psB[:, :NDIR])
        nc.scalar.dma_start(out=z[:, NPK:], in_=out_b[:])
    nc.finalize()
    return nc


def _get_program():
    if "nc" not in _PROGRAM_CACHE:
        _PROGRAM_CACHE["nc"] = build_program()
    return _PROGRAM_CACHE["nc"]


# ---------------------------------------------------------------- entry
def kernel(**inputs):
    from concourse.bass_utils import run_bass_kernel_spmd

    in_maps, corr = _prepare(inputs)
    nc = _get_program()
    res = run_bass_kernel_spmd(nc, in_maps, list(range(NCORES)))
    out_reorder = np.asarray(inputs["out_reorder"], np.int64)
    # per-core: col j<NPK is packed field r=j//WPF (descale 8^-r, add corr);
    # col j>=NPK is direct (exact, no correction)
    rs = np.concatenate([np.repeat(8.0 ** -np.arange(NR), WPF),
                         np.ones(NDIR, np.float64)]).astype(np.float32)
    y = np.empty((T, OUT_F), np.float32)
    for c in range(NCORES):
        zc = res.results[c]["z"].astype(np.float32) * rs[None, :]
        zc[:, :NPK] += corr[:, OPC * c:OPC * c + NPK]
        y[:, OPC * c:OPC * (c + 1)] = zc
    y = y[:, out_reorder]
    return y.reshape(1, T, OUT_F).astype(np.float32)
